# revision 18
# baseline (speedup 1.0000x reference)
"""Trainium2 Bass kernel for CustomScaledDotProductAttention.

Full module: y = out_proj(softmax(q k^T / sqrt(D)) v) with fused qkv proj.
Shapes: x [2, 2048, 1024], H=16 heads, D=64.

Sharding (8 cores): core = b*4 + g, b = batch (2), g = head-group (4 heads).
Each core computes its batch's qkv projection restricted to its 4 heads,
attention for those heads, and the out-proj partial product.  Host sums the
4 partials per batch and adds b_out.

Design vs the fp32r baseline (457us):
  - The attention phase of the baseline ran entirely at the HAM cold clock
    (K=4/8, 1.2 GHz): fp32r matmuls self-load weights, which serializes the
    row-tiled score pairs, and the half-array matmuls (K=64 scores, M=65
    attn@v) never generate enough PE activity to release the clock gate.
  - All attention operands are bf16: row-tiled score pairs (tile_position
    (0,0)/(64,0)) can run concurrently, LDWEIGHTS gets FWL, DMA halves.
  - attn@v stationary padded to the full 128 columns (head's V + ones col +
    neighbor head data) so every matmul exercises the whole PE array.
  - LAG-1 software pipeline: scores(m+1) is emitted before attn@v(m), so
    the PE runs a step ahead of the ScalarE exp stream (the pacer).
  - exp reads a 2-bank PSUM tile ([128,2,512]) per m-step: one ACT per
    m-step instead of two.
  - softmax normalization: ones-column gives l in psum row 64; batched
    reciprocal_approx_fast on [2,512] (the baseline burned 3.3us per
    1-partition reciprocal); broadcast via DRAM bounce; multiply on DVE.
  - qk-proj for the second head pair is interleaved into the p=0 attention
    blocks (1 matmul per m-step), out-proj t-blocks into the p=1 blocks:
    no serial projection phases, PE bubbles get filled, y DMA is spread.
"""

import numpy as np

import concourse.bass as bass
import concourse.mybir as mybir
import concourse.tile as tile
from concourse import library_config
from concourse.bass_utils import run_bass_kernel_spmd

F32 = mybir.dt.float32
BF16 = mybir.dt.bfloat16

B, N, C, H, D = 2, 2048, 1024, 16, 64
SCALE = D ** -0.5          # 0.125
HPC = 4                    # heads per core
N_CORES = 8
NK = C // 128              # 8 contraction chunks of 128
NM = N // 128              # 16 m-chunks (key blocks)
NN = N // 512              # 4 n-chunks (query blocks of 512)
VW = HPC * (D + 1)         # 260: v columns + ones column per head
VWP = VW + 63              # 323: padded so every head has 128 cols to load


def _emit(tc, nc, xT, wqk, bq, wv, wo, y, lbounce):
    PS = bass.MemorySpace.PSUM
    EXP = mybir.ActivationFunctionType.Exp

    with (
        nc.allow_low_precision(reason="bf16 attention operands; psum f32"),
        tc.tile_pool(name="persist", bufs=1) as pp,
        tc.tile_pool(name="qk", bufs=1) as qkp,
        tc.tile_pool(name="vp", bufs=1) as vp,
        tc.tile_pool(name="at", bufs=1) as atp,
        tc.tile_pool(name="xw", bufs=1) as xw,
    ):
        # ---- persistent tiles ----
        QK = [qkp.tile([128, N], BF16, tag=f"qk{j}", name=f"qk{j}")
              for j in range(4)]
        Vb = vp.tile([128, NM, VWP], BF16, tag="vb")   # V[m] = Vb[:, m, :]
        AT = [atp.tile([128, N], BF16, tag=f"at{p}", name=f"at{p}")
              for p in range(2)]
        wo0 = pp.tile([128, 1024], BF16, tag="wo0")
        wo1 = pp.tile([128, 1024], BF16, tag="wo1")
        bqs = pp.tile([128, 4], F32, tag="bqs")

        # chunked loads so matmuls can start before the full input lands;
        # first chunk's x and qk-weights lead, on separate queues
        xt = xw.tile([128, NK, N], BF16, tag="xt")
        wq = xw.tile([128, NK, 512], BF16, tag="wq")
        wvt = xw.tile([128, NK, VWP], BF16, tag="wvt")
        for c in range(NK):
            nc.sync.dma_start(out=xt[:, c, :],
                              in_=xT[c * 128:(c + 1) * 128, :])
            nc.gpsimd.dma_start(out=wq[:, c, :],
                                in_=wqk[c * 128:(c + 1) * 128, :])
        nc.gpsimd.dma_start(out=bqs, in_=bq[:, :])
        for c in range(NK):
            nc.sync.dma_start(out=wvt[:, c, :],
                              in_=wv[c * 128:(c + 1) * 128, :])
        xt1f = xw.tile([1, N], F32, tag="xt1f")
        nc.vector.memset(xt1f, 1.0)
        xt1 = xw.tile([1, N], BF16, tag="xt1")
        nc.vector.tensor_copy(xt1, xt1f)
        wvb = xw.tile([1, VWP], BF16, tag="wvb")
        nc.gpsimd.dma_start(out=wvb, in_=wv[C:C + 1, :])
        # wo loads early; consumed only by the interleaved out-proj
        nc.sync.dma_start(out=wo0, in_=wo[0:128, :])
        nc.sync.dma_start(out=wo1, in_=wo[128:256, :])

        # ========== phase 1: qk-proj (head pair 0) + v-proj ==========
        with tc.tile_pool(name="pps", bufs=8, space=PS) as pps:
            # qk-proj j=0,1 only (heads 0/1 q and k); j=2,3 is interleaved
            # into the p=0 attention blocks later.  Contraction-chunk OUTER
            # so compute starts as soon as DMA chunk 0 lands.
            ps = [[pps.tile([128, 512], F32, tag="pq", name="pq")
                   for n in range(NN)] for j in range(2)]
            for c in range(NK):
                for j in range(2):
                    for n in range(NN):
                        nc.tensor.matmul(
                            ps[j][n], wq[:, c, j * 128:(j + 1) * 128],
                            xt[:, c, n * 512:(n + 1) * 512],
                            start=(c == 0), stop=(c == NK - 1))
            for j in range(2):
                for n in range(NN):
                    nc.vector.tensor_scalar_add(
                        QK[j][:, n * 512:(n + 1) * 512], ps[j][n],
                        bqs[:, j:j + 1])

            # v-proj: V[m] = sum_c xt[c][:, m].T @ wvt[c] (+bias row)
            for m in range(NM):
                psv = pps.tile([128, VWP], F32, tag="pq", name="pv")
                for c in range(NK):
                    nc.tensor.matmul(
                        psv, xt[:, c, m * 128:(m + 1) * 128], wvt[:, c, :],
                        start=(c == 0), stop=False)
                nc.tensor.matmul(
                    psv, xt1[:, m * 128:(m + 1) * 128], wvb,
                    start=False, stop=True)
                nc.scalar.copy(Vb[:, m, :], psv)

        # ========== phase 2: attention (+ interleaved proj work) ==========
        with (
            tc.tile_pool(name="sc", bufs=2, space=PS) as scp,
            tc.tile_pool(name="ot", bufs=2, space=PS) as otp,
            tc.tile_pool(name="et", bufs=3) as etp,
            tc.tile_pool(name="lv", bufs=4) as lvp,
            tc.tile_pool(name="orw", bufs=4) as orp,
            tc.tile_pool(name="bcs", bufs=4) as bcp,
            tc.tile_pool(name="ysb", bufs=2) as ysbp,
        ):
            def emit_sp(p, n, m):
                Qt, Kt = QK[2 * p], QK[2 * p + 1]
                ncol = slice(n * 512, (n + 1) * 512)
                sp = scp.tile([128, 2, 512], F32, tag="sp", name="sp")
                nc.tensor.matmul(
                    sp[:, 0, :], Kt[0:64, m * 128:(m + 1) * 128],
                    Qt[0:64, ncol],
                    start=True, stop=True, tile_position=(0, 0))
                nc.tensor.matmul(
                    sp[:, 1, :], Kt[64:128, m * 128:(m + 1) * 128],
                    Qt[64:128, ncol],
                    start=True, stop=True, tile_position=(64, 0))
                return sp

            def attention_block(p, n, sp0, next_pn=None, extras=None,
                                post=None, fast_epi_pool=None):
                """One (head-pair, query-chunk) block, LAG-1 pipelined.
                sp0: pre-emitted first score tile (cross-block preload).
                next_pn: next block's (p, n) — its first score pair is
                emitted before this block's last attn@v so the PE never
                drains at block boundaries.  Returns the preloaded tile.
                extras: optional list of 16 callables, one run per m-step
                (used to slot projection/out-proj matmuls into PE bubbles).
                post: emitted after the m-loop, before the epilogue."""
                ncol = slice(n * 512, (n + 1) * 512)
                ot = [otp.tile([128, 512], F32, tag="ot", name="ot")
                      for _ in range(2)]

                sp_cur, nxt_sp0 = sp0, None
                for m in range(NM):
                    e = etp.tile([128, 2, 512], BF16, tag="et")
                    nc.scalar.activation(e, sp_cur, EXP, scale=SCALE)
                    if m + 1 < NM:
                        sp_nxt = emit_sp(p, n, m + 1)
                    elif next_pn is not None:
                        nxt_sp0 = emit_sp(next_pn[0], next_pn[1], 0)
                    for hh in range(2):
                        c0 = (2 * p + hh) * (D + 1)
                        nc.tensor.matmul(
                            ot[hh], Vb[:, m, c0:c0 + 128], e[:, hh, :],
                            start=(m == 0), stop=(m == NM - 1))
                    if extras is not None and extras[m] is not None:
                        extras[m]()
                    if m + 1 < NM:
                        sp_cur = sp_nxt
                if post is not None:
                    post()

                ncol_ = ncol
                if fast_epi_pool is not None:
                    # final block: skip the DRAM transpose/broadcast round
                    # trips — reciprocal straight from the psum l-rows, and
                    # broadcast with K=1 matmuls on the (now idle) PE.
                    linvs = []
                    for hh in range(2):
                        li = lvp.tile([1, 512], BF16, tag="lv", name="lis")
                        nc.vector.reciprocal(li, ot[hh][64:65, :])
                        linvs.append(li)
                    orw = []
                    for hh in range(2):
                        o = orp.tile([65, 512], F32, tag="orw", name="orw")
                        nc.vector.tensor_copy(o, ot[hh][0:65, :])
                        orw.append(o)
                    bcps = fast_epi_pool.tile([128, 2, 512], F32,
                                              tag="yp", name="bcps")
                    for hh in range(2):
                        nc.tensor.matmul(
                            bcps[0:64, hh, :], xt1[0:1, 0:64], linvs[hh],
                            start=True, stop=True)
                        nc.vector.tensor_mul(
                            AT[p][hh * 64:(hh + 1) * 64, ncol_],
                            orw[hh][0:64, :], bcps[0:64, hh, :])
                    return nxt_sp0

                # epilogue: evacuate psum, batched recip, normalize into AT
                orw = []
                for hh in range(2):
                    o = orp.tile([65, 512], F32, tag="orw", name="orw")
                    nc.vector.tensor_copy(o, ot[hh][0:65, :])
                    orw.append(o)
                # reshape l rows onto 128 partitions (SBUF->SBUF DMA) so the
                # DVE reciprocal runs ~50x faster than on a 1-partition AP
                idx = (p * NN + n) * 2
                lt = lvp.tile([128, 8], F32, tag="lv", name="lt")
                nc.sync.dma_start(out=lt[0:64, :], in_=orw[0][64:65, :])
                nc.sync.dma_start(out=lt[64:128, :], in_=orw[1][64:65, :])
                linvt = lvp.tile([128, 8], F32, tag="lv", name="lit")
                nc.vector.reciprocal(linvt, lt)
                nc.sync.dma_start(out=lbounce[idx:idx + 2, :], in_=linvt)
                for hh in range(2):
                    bc = bcp.tile([64, 512], F32, tag="bc", name="bc")
                    nc.sync.dma_start(
                        out=bc,
                        in_=lbounce[idx + hh:idx + hh + 1, :]
                        .to_broadcast((64, 512)))
                    nc.vector.tensor_mul(
                        AT[p][hh * 64:(hh + 1) * 64, ncol],
                        orw[hh][0:64, :], bc)
                return nxt_sp0

            # ---- p=0 blocks with qk-proj j=2,3 interleaved ----
            with tc.tile_pool(name="qq", bufs=2, space=PS) as qqp:
                sp0 = emit_sp(0, 0, 0)
                for n in range(NN):
                    pq = [qqp.tile([128, 512], F32, tag="q2", name="q2")
                          for _ in range(2)]

                    def mk_qk(c, j2, pq=pq, n=n):
                        def go():
                            j = 2 + j2
                            nc.tensor.matmul(
                                pq[j2], wq[:, c, j * 128:(j + 1) * 128],
                                xt[:, c, n * 512:(n + 1) * 512],
                                start=(c == 0), stop=(c == NK - 1))
                        return go

                    extras = [mk_qk(m // 2, m % 2) for m in range(NM)]

                    def post(pq=pq, n=n):
                        for j2 in range(2):
                            nc.vector.tensor_scalar_add(
                                QK[2 + j2][:, n * 512:(n + 1) * 512],
                                pq[j2], bqs[:, 2 + j2:3 + j2])

                    nxt = (0, n + 1) if n + 1 < NN else (1, 0)
                    sp0 = attention_block(0, n, sp0, next_pn=nxt,
                                          extras=extras, post=post)

            # ---- p=1 blocks with out-proj t-blocks interleaved ----
            with tc.tile_pool(name="yps", bufs=1, space=PS) as ypsp:
                def mk_outproj_steps(t):
                    """4 matmuls + evacuation for output t-block t,
                    split into 4 callables."""
                    state = {}

                    def step(sub, t=t):
                        def go():
                            if sub == 0:
                                state["yp"] = ypsp.tile(
                                    [128, 2, 512], F32, tag="yp", name="yp")
                            yp = state["yp"]
                            ic, oc = sub // 2, sub % 2
                            a = AT[ic]
                            w = wo0 if ic == 0 else wo1
                            nc.tensor.matmul(
                                yp[:, oc, :], a[:, t * 128:(t + 1) * 128],
                                w[:, oc * 512:(oc + 1) * 512],
                                start=(ic == 0), stop=(ic == 1))
                            if sub == 3:
                                ysb = ysbp.tile([128, 1024], BF16,
                                                tag="ysb")
                                nc.vector.tensor_copy(
                                    ysb[:, 0:512], yp[:, 0, :])
                                nc.vector.tensor_copy(
                                    ysb[:, 512:1024], yp[:, 1, :])
                                nc.sync.dma_start(
                                    out=y[t * 128:(t + 1) * 128, :],
                                    in_=ysb)
                        return go
                    return [step(s) for s in range(4)]

                for n in range(NN):
                    extras = [None] * NM
                    if n >= 1:
                        steps = []
                        for t in range(4 * (n - 1), 4 * n):
                            steps.extend(mk_outproj_steps(t))
                        # 2 sub-steps per m-step on m=6..13: the first
                        # AT[1]-reading matmul lands ~7us into the block,
                        # after the previous block's normalization chain.
                        def pair(a, b):
                            def go():
                                a()
                                b()
                            return go
                        extras = [None] * NM
                        for i in range(8):
                            extras[6 + i] = pair(steps[2 * i],
                                                 steps[2 * i + 1])
                    nxt = (1, n + 1) if n + 1 < NN else None
                    sp0 = attention_block(
                        1, n, sp0, next_pn=nxt, extras=extras,
                        fast_epi_pool=(ypsp if nxt is None else None))

        # ---- tail: last 4 t-blocks in their own deeper psum pool ----
        with (
            tc.tile_pool(name="ytp", bufs=3, space=PS) as ytp,
            tc.tile_pool(name="ysbt", bufs=3) as ysbt,
        ):
            for t in range(12, 16):
                yp = ytp.tile([128, 2, 512], F32, tag="yt", name="yt")
                for ic in range(2):
                    a, w = AT[ic], (wo0 if ic == 0 else wo1)
                    for oc in range(2):
                        nc.tensor.matmul(
                            yp[:, oc, :], a[:, t * 128:(t + 1) * 128],
                            w[:, oc * 512:(oc + 1) * 512],
                            start=(ic == 0), stop=(ic == 1))
                ysb = ysbt.tile([128, 1024], BF16, tag="ysbt")
                nc.scalar.copy(ysb[:, 0:512], yp[:, 0, :])
                nc.scalar.copy(ysb[:, 512:1024], yp[:, 1, :])
                nc.sync.dma_start(out=y[t * 128:(t + 1) * 128, :],
                                  in_=ysb)


def _split_multi_waits(nc):
    """Hoist all-but-one sem wait from instructions onto standalone
    EventSemaphore instructions: most TRN2 instruction encodings carry a
    single sync-wait slot (walrus: "Too many sync wait commands")."""
    import bass_rust
    nop_id = [0]
    for fn in nc.m.functions:
        for blk in fn.blocks:
            insts = blk.instructions
            out = []
            changed = False
            for ins in insts:
                si = ins.sync_info
                is_evsem = isinstance(ins, mybir.InstEventSemaphore)
                if (si is not None and si.on_wait is not None
                        and len(si.on_wait) > 1 and not is_evsem):
                    waits = list(si.on_wait)
                    for w in waits[:-1]:
                        ev = mybir.InstEventSemaphore(
                            name=f"waitev_{nop_id[0]}", engine=ins.engine)
                        nop_id[0] += 1
                        ev.sync_info = bass_rust.SyncInfo(
                            on_wait=[w], on_update=[])
                        out.append(ev)
                    ins.sync_info = bass_rust.SyncInfo(
                        on_wait=[waits[-1]],
                        on_update=list(si.on_update or []))
                    changed = True
                out.append(ins)
            if changed:
                blk.instructions = out


def build_bass(split_waits=True):
    nc = bass.Bass()
    xT = nc.dram_tensor("xT", [C, N], BF16, kind="ExternalInput")
    wqk = nc.dram_tensor("wqk", [C, 512], BF16, kind="ExternalInput")
    bq = nc.dram_tensor("bq", [128, 4], F32, kind="ExternalInput")
    wv = nc.dram_tensor("wv", [C + 1, VWP], BF16, kind="ExternalInput")
    wo = nc.dram_tensor("wo", [2 * 128, 1024], BF16, kind="ExternalInput")
    y = nc.dram_tensor("y", [N, C], BF16, kind="ExternalOutput")
    lbounce = nc.dram_tensor("lbounce", [16, 512], F32)
    with tile.TileContext(nc) as tc:
        _emit(tc, nc, xT, wqk, bq, wv, wo, y, lbounce)
    if split_waits:
        _split_multi_waits(nc)
    return nc


def prep_core_inputs(x, w_qkv, b_qkv, w_out, core):
    """Build the per-core input arrays (bf16 for all matmul operands)."""
    import ml_dtypes
    bf = ml_dtypes.bfloat16
    b, g = divmod(core, HPC)
    heads = [HPC * g + i for i in range(HPC)]
    f = np.float32

    xTa = np.ascontiguousarray(x[b].T, f)

    def q_rows(h):
        return w_qkv[h * D:(h + 1) * D]

    def k_rows(h):
        return w_qkv[C + h * D:C + (h + 1) * D]

    def v_rows(h):
        return w_qkv[2 * C + h * D:2 * C + (h + 1) * D]

    h0, h1, h2, h3 = heads
    wqk_rows = np.concatenate([
        q_rows(h0), q_rows(h1), k_rows(h0), k_rows(h1),
        q_rows(h2), q_rows(h3), k_rows(h2), k_rows(h3)], 0)   # [512, C]
    bqk = np.concatenate([
        b_qkv[h0 * D:(h0 + 1) * D], b_qkv[h1 * D:(h1 + 1) * D],
        b_qkv[C + h0 * D:C + (h0 + 1) * D],
        b_qkv[C + h1 * D:C + (h1 + 1) * D],
        b_qkv[h2 * D:(h2 + 1) * D], b_qkv[h3 * D:(h3 + 1) * D],
        b_qkv[C + h2 * D:C + (h2 + 1) * D],
        b_qkv[C + h3 * D:C + (h3 + 1) * D]], 0)               # [512]

    wv_aug = np.zeros((C + 1, VWP), f)
    for i, h in enumerate(heads):
        wv_aug[:C, i * (D + 1):i * (D + 1) + D] = v_rows(h).T
        wv_aug[C, i * (D + 1):i * (D + 1) + D] = \
            b_qkv[2 * C + h * D:2 * C + (h + 1) * D]
        wv_aug[C, i * (D + 1) + D] = 1.0
    # pad columns so head 3's 128-col stationary slice stays in range;
    # the products land in unused psum partitions.
    wv_aug[:, VW:] = wv_aug[:, :VWP - VW]

    woa = np.concatenate([w_out[:, h * D:(h + 1) * D].T for h in heads], 0)

    return {
        "xT": np.ascontiguousarray(xTa).astype(bf),
        "wqk": np.ascontiguousarray(wqk_rows.T, f).astype(bf),
        "bq": np.ascontiguousarray(bqk.reshape(4, 128).T, f),
        "wv": np.ascontiguousarray(wv_aug).astype(bf),
        "wo": np.ascontiguousarray(woa).astype(bf),
    }


def assemble_output(partials, b_out):
    """partials: list of 8 [N, C] arrays (core order). Returns [B, N, C]."""
    y = np.empty((B, N, C), np.float32)
    for b in range(B):
        acc = partials[HPC * b].astype(np.float32)
        for g in range(1, HPC):
            acc = acc + partials[HPC * b + g].astype(np.float32)
        y[b] = acc + b_out.astype(np.float32)
    return y


_NC_CACHE = {}


def run(inputs, trace=False):
    """Returns (y_full [B,N,C] f32, exec_time_ns or None)."""
    x = np.asarray(inputs["x"], np.float32)
    w_qkv = np.asarray(inputs["w_qkv"], np.float32)
    b_qkv = np.asarray(inputs["b_qkv"], np.float32)
    w_out = np.asarray(inputs["w_out"], np.float32)
    b_out = np.asarray(inputs["b_out"], np.float32)

    if "nc" not in _NC_CACHE:
        _NC_CACHE["nc"] = build_bass()
    nc = _NC_CACHE["nc"]

    in_maps = [prep_core_inputs(x, w_qkv, b_qkv, w_out, core)
               for core in range(N_CORES)]
    res = run_bass_kernel_spmd(nc, in_maps, list(range(N_CORES)),
                               trace=trace)
    partials = [res.results[i]["y"] for i in range(N_CORES)]
    return assemble_output(partials, b_out), res.exec_time_ns


def kernel(**inputs):
    y, _ = run(inputs, trace=False)
    return y


# revision 19
# speedup vs baseline: 1.2070x; 1.2070x over previous
"""Trainium2 Bass kernel for CustomScaledDotProductAttention.

Full module: y = out_proj(softmax(q k^T / sqrt(D)) v) with fused qkv proj.
Shapes: x [2, 2048, 1024], H=16 heads, D=64.

Sharding (8 cores): core = b*4 + g, b = batch (2), g = head-group (4 heads).
Each core computes its batch's qkv projection restricted to its 4 heads,
attention for those heads, and the out-proj partial product.  Host sums the
4 partials per batch and adds b_out.

Design vs the fp32r baseline (457us):
  - The attention phase of the baseline ran entirely at the HAM cold clock
    (K=4/8, 1.2 GHz): fp32r matmuls self-load weights, which serializes the
    row-tiled score pairs, and the half-array matmuls (K=64 scores, M=65
    attn@v) never generate enough PE activity to release the clock gate.
  - All attention operands are bf16: row-tiled score pairs (tile_position
    (0,0)/(64,0)) can run concurrently, LDWEIGHTS gets FWL, DMA halves.
  - attn@v stationary padded to the full 128 columns (head's V + ones col +
    neighbor head data) so every matmul exercises the whole PE array.
  - LAG-1 software pipeline: scores(m+1) is emitted before attn@v(m), so
    the PE runs a step ahead of the ScalarE exp stream (the pacer).
  - exp reads a 2-bank PSUM tile ([128,2,512]) per m-step: one ACT per
    m-step instead of two.
  - softmax normalization: ones-column gives l in psum row 64; batched
    reciprocal_approx_fast on [2,512] (the baseline burned 3.3us per
    1-partition reciprocal); broadcast via DRAM bounce; multiply on DVE.
  - qk-proj for the second head pair is interleaved into the p=0 attention
    blocks (1 matmul per m-step), out-proj t-blocks into the p=1 blocks:
    no serial projection phases, PE bubbles get filled, y DMA is spread.
"""

import numpy as np

import concourse.bass as bass
import concourse.mybir as mybir
import concourse.tile as tile
from concourse import library_config
from concourse.bass_utils import run_bass_kernel_spmd

F32 = mybir.dt.float32
BF16 = mybir.dt.bfloat16

B, N, C, H, D = 2, 2048, 1024, 16, 64
SCALE = D ** -0.5          # 0.125
HPC = 4                    # heads per core
N_CORES = 8
NK = C // 128              # 8 contraction chunks of 128
NM = N // 128              # 16 m-chunks (key blocks)
NN = N // 512              # 4 n-chunks (query blocks of 512)
VW = HPC * (D + 1)         # 260: v columns + ones column per head
VWP = VW + 63              # 323: padded so every head has 128 cols to load


def _emit(tc, nc, xT, wqk, bq, wv, wo, y, lbounce):
    PS = bass.MemorySpace.PSUM
    EXP = mybir.ActivationFunctionType.Exp

    with (
        nc.allow_low_precision(reason="bf16 attention operands; psum f32"),
        tc.tile_pool(name="persist", bufs=1) as pp,
        tc.tile_pool(name="qk", bufs=1) as qkp,
        tc.tile_pool(name="vp", bufs=1) as vp,
        tc.tile_pool(name="at", bufs=1) as atp,
        tc.tile_pool(name="xw", bufs=1) as xw,
    ):
        # ---- persistent tiles ----
        QK = [qkp.tile([128, N], BF16, tag=f"qk{j}", name=f"qk{j}")
              for j in range(4)]
        Vb = vp.tile([128, NM, VWP], BF16, tag="vb")   # V[m] = Vb[:, m, :]
        AT = [atp.tile([128, N], BF16, tag=f"at{p}", name=f"at{p}")
              for p in range(2)]
        wo0 = pp.tile([128, 1024], BF16, tag="wo0")
        wo1 = pp.tile([128, 1024], BF16, tag="wo1")
        bqs = pp.tile([128, 4], F32, tag="bqs")

        # chunked loads so matmuls can start before the full input lands;
        # first chunk's x and qk-weights lead, on separate queues
        xt = xw.tile([128, NK, N], BF16, tag="xt")
        wq = xw.tile([128, NK, 512], BF16, tag="wq")
        wvt = xw.tile([128, NK, VWP], BF16, tag="wvt")
        for c in range(NK):
            nc.sync.dma_start(out=xt[:, c, :],
                              in_=xT[c * 128:(c + 1) * 128, :])
            nc.gpsimd.dma_start(out=wq[:, c, :],
                                in_=wqk[c * 128:(c + 1) * 128, :])
        nc.gpsimd.dma_start(out=bqs, in_=bq[:, :])
        for c in range(NK):
            nc.sync.dma_start(out=wvt[:, c, :],
                              in_=wv[c * 128:(c + 1) * 128, :])
        xt1f = xw.tile([1, N], F32, tag="xt1f")
        nc.vector.memset(xt1f, 1.0)
        xt1 = xw.tile([1, N], BF16, tag="xt1")
        nc.vector.tensor_copy(xt1, xt1f)
        wvb = xw.tile([1, VWP], BF16, tag="wvb")
        nc.gpsimd.dma_start(out=wvb, in_=wv[C:C + 1, :])
        # wo loads early; consumed only by the interleaved out-proj
        nc.sync.dma_start(out=wo0, in_=wo[0:128, :])
        nc.sync.dma_start(out=wo1, in_=wo[128:256, :])

        # ========== phase 1: qk-proj (head pair 0) + v-proj ==========
        with tc.tile_pool(name="pps", bufs=8, space=PS) as pps:
            # qk-proj j=0,1 only (heads 0/1 q and k); j=2,3 is interleaved
            # into the p=0 attention blocks later.  Contraction-chunk OUTER
            # so compute starts as soon as DMA chunk 0 lands.
            ps = [[pps.tile([128, 512], F32, tag="pq", name="pq")
                   for n in range(NN)] for j in range(2)]
            for c in range(NK):
                for j in range(2):
                    for n in range(NN):
                        nc.tensor.matmul(
                            ps[j][n], wq[:, c, j * 128:(j + 1) * 128],
                            xt[:, c, n * 512:(n + 1) * 512],
                            start=(c == 0), stop=(c == NK - 1))
            for j in range(2):
                for n in range(NN):
                    nc.vector.tensor_scalar_add(
                        QK[j][:, n * 512:(n + 1) * 512], ps[j][n],
                        bqs[:, j:j + 1])

            # v-proj: V[m] = sum_c xt[c][:, m].T @ wvt[c] (+bias row)
            for m in range(NM):
                psv = pps.tile([128, VWP], F32, tag="pq", name="pv")
                for c in range(NK):
                    nc.tensor.matmul(
                        psv, xt[:, c, m * 128:(m + 1) * 128], wvt[:, c, :],
                        start=(c == 0), stop=False)
                nc.tensor.matmul(
                    psv, xt1[:, m * 128:(m + 1) * 128], wvb,
                    start=False, stop=True)
                nc.scalar.copy(Vb[:, m, :], psv)

        # ========== phase 2: attention (+ interleaved proj work) ==========
        with (
            tc.tile_pool(name="sc", bufs=2, space=PS) as scp,
            tc.tile_pool(name="ot", bufs=2, space=PS) as otp,
            tc.tile_pool(name="et", bufs=3) as etp,
            tc.tile_pool(name="lv", bufs=4) as lvp,
            tc.tile_pool(name="orw", bufs=4) as orp,
            tc.tile_pool(name="bcs", bufs=4) as bcp,
            tc.tile_pool(name="ysb", bufs=2) as ysbp,
        ):
            def emit_sp(p, n, m):
                Qt, Kt = QK[2 * p], QK[2 * p + 1]
                ncol = slice(n * 512, (n + 1) * 512)
                sp = scp.tile([128, 2, 512], F32, tag="sp", name="sp")
                nc.tensor.matmul(
                    sp[:, 0, :], Kt[0:64, m * 128:(m + 1) * 128],
                    Qt[0:64, ncol],
                    start=True, stop=True, tile_position=(0, 0))
                nc.tensor.matmul(
                    sp[:, 1, :], Kt[64:128, m * 128:(m + 1) * 128],
                    Qt[64:128, ncol],
                    start=True, stop=True, tile_position=(64, 0))
                return sp

            def attention_block(p, n, sp0, next_pn=None, extras=None,
                                post=None, fast_epi_pool=None):
                """One (head-pair, query-chunk) block, LAG-1 pipelined.
                sp0: pre-emitted first score tile (cross-block preload).
                next_pn: next block's (p, n) — its first score pair is
                emitted before this block's last attn@v so the PE never
                drains at block boundaries.  Returns the preloaded tile.
                extras: optional list of 16 callables, one run per m-step
                (used to slot projection/out-proj matmuls into PE bubbles).
                post: emitted after the m-loop, before the epilogue."""
                ncol = slice(n * 512, (n + 1) * 512)
                ot = [otp.tile([128, 512], F32, tag="ot", name="ot")
                      for _ in range(2)]

                sp_cur, nxt_sp0 = sp0, None
                for m in range(NM):
                    e = etp.tile([128, 2, 512], BF16, tag="et")
                    nc.scalar.activation(e, sp_cur, EXP, scale=SCALE)
                    if m + 1 < NM:
                        sp_nxt = emit_sp(p, n, m + 1)
                    elif next_pn is not None:
                        nxt_sp0 = emit_sp(next_pn[0], next_pn[1], 0)
                    for hh in range(2):
                        c0 = (2 * p + hh) * (D + 1)
                        nc.tensor.matmul(
                            ot[hh], Vb[:, m, c0:c0 + 128], e[:, hh, :],
                            start=(m == 0), stop=(m == NM - 1))
                    if extras is not None and extras[m] is not None:
                        extras[m]()
                    if m + 1 < NM:
                        sp_cur = sp_nxt
                if post is not None:
                    post()

                # epilogue: evacuate psum, batched recip, normalize into AT
                orw = []
                for hh in range(2):
                    o = orp.tile([65, 512], F32, tag="orw", name="orw")
                    nc.vector.tensor_copy(o, ot[hh][0:65, :])
                    orw.append(o)
                # reshape l rows onto 128 partitions (SBUF->SBUF DMA) so the
                # DVE reciprocal runs ~50x faster than on a 1-partition AP
                idx = (p * NN + n) * 2
                lt = lvp.tile([128, 8], F32, tag="lv", name="lt")
                nc.sync.dma_start(out=lt[0:64, :], in_=orw[0][64:65, :])
                nc.sync.dma_start(out=lt[64:128, :], in_=orw[1][64:65, :])
                linvt = lvp.tile([128, 8], F32, tag="lv", name="lit")
                nc.vector.reciprocal(linvt, lt)
                nc.sync.dma_start(out=lbounce[idx:idx + 2, :], in_=linvt)
                for hh in range(2):
                    bc = bcp.tile([64, 512], F32, tag="bc", name="bc")
                    nc.sync.dma_start(
                        out=bc,
                        in_=lbounce[idx + hh:idx + hh + 1, :]
                        .to_broadcast((64, 512)))
                    nc.vector.tensor_mul(
                        AT[p][hh * 64:(hh + 1) * 64, ncol],
                        orw[hh][0:64, :], bc)
                return nxt_sp0

            # ---- p=0 blocks with qk-proj j=2,3 interleaved ----
            with tc.tile_pool(name="qq", bufs=2, space=PS) as qqp:
                sp0 = emit_sp(0, 0, 0)
                for n in range(NN):
                    pq = [qqp.tile([128, 512], F32, tag="q2", name="q2")
                          for _ in range(2)]

                    def mk_qk(c, j2, pq=pq, n=n):
                        def go():
                            j = 2 + j2
                            nc.tensor.matmul(
                                pq[j2], wq[:, c, j * 128:(j + 1) * 128],
                                xt[:, c, n * 512:(n + 1) * 512],
                                start=(c == 0), stop=(c == NK - 1))
                        return go

                    extras = [mk_qk(m // 2, m % 2) for m in range(NM)]

                    def post(pq=pq, n=n):
                        for j2 in range(2):
                            nc.vector.tensor_scalar_add(
                                QK[2 + j2][:, n * 512:(n + 1) * 512],
                                pq[j2], bqs[:, 2 + j2:3 + j2])

                    nxt = (0, n + 1) if n + 1 < NN else (1, 0)
                    sp0 = attention_block(0, n, sp0, next_pn=nxt,
                                          extras=extras, post=post)

            # ---- p=1 blocks with out-proj t-blocks interleaved ----
            with tc.tile_pool(name="yps", bufs=1, space=PS) as ypsp:
                def mk_outproj_steps(t):
                    """4 matmuls + evacuation for output t-block t,
                    split into 4 callables."""
                    state = {}

                    def step(sub, t=t):
                        def go():
                            if sub == 0:
                                state["yp"] = ypsp.tile(
                                    [128, 2, 512], F32, tag="yp", name="yp")
                            yp = state["yp"]
                            ic, oc = sub // 2, sub % 2
                            a = AT[ic]
                            w = wo0 if ic == 0 else wo1
                            nc.tensor.matmul(
                                yp[:, oc, :], a[:, t * 128:(t + 1) * 128],
                                w[:, oc * 512:(oc + 1) * 512],
                                start=(ic == 0), stop=(ic == 1))
                            if sub == 3:
                                ysb = ysbp.tile([128, 1024], BF16,
                                                tag="ysb")
                                nc.vector.tensor_copy(
                                    ysb[:, 0:512], yp[:, 0, :])
                                nc.vector.tensor_copy(
                                    ysb[:, 512:1024], yp[:, 1, :])
                                nc.sync.dma_start(
                                    out=y[t * 128:(t + 1) * 128, :],
                                    in_=ysb)
                        return go
                    return [step(s) for s in range(4)]

                for n in range(NN):
                    extras = [None] * NM
                    if n >= 1:
                        steps = []
                        for t in range(4 * (n - 1), 4 * n):
                            steps.extend(mk_outproj_steps(t))
                        # 2 sub-steps per m-step on m=6..13: the first
                        # AT[1]-reading matmul lands ~7us into the block,
                        # after the previous block's normalization chain.
                        def pair(a, b):
                            def go():
                                a()
                                b()
                            return go
                        extras = [None] * NM
                        for i in range(8):
                            extras[6 + i] = pair(steps[2 * i],
                                                 steps[2 * i + 1])
                    nxt = (1, n + 1) if n + 1 < NN else None
                    sp0 = attention_block(
                        1, n, sp0, next_pn=nxt, extras=extras,
                        fast_epi_pool=(ypsp if nxt is None else None))

        # ---- tail: last 4 t-blocks in their own deeper psum pool ----
        with (
            tc.tile_pool(name="ytp", bufs=3, space=PS) as ytp,
            tc.tile_pool(name="ysbt", bufs=3) as ysbt,
        ):
            for t in range(12, 16):
                yp = ytp.tile([128, 2, 512], F32, tag="yt", name="yt")
                for ic in range(2):
                    a, w = AT[ic], (wo0 if ic == 0 else wo1)
                    for oc in range(2):
                        nc.tensor.matmul(
                            yp[:, oc, :], a[:, t * 128:(t + 1) * 128],
                            w[:, oc * 512:(oc + 1) * 512],
                            start=(ic == 0), stop=(ic == 1))
                ysb = ysbt.tile([128, 1024], BF16, tag="ysbt")
                nc.scalar.copy(ysb[:, 0:512], yp[:, 0, :])
                nc.scalar.copy(ysb[:, 512:1024], yp[:, 1, :])
                nc.sync.dma_start(out=y[t * 128:(t + 1) * 128, :],
                                  in_=ysb)


def _split_multi_waits(nc):
    """Hoist all-but-one sem wait from instructions onto standalone
    EventSemaphore instructions: most TRN2 instruction encodings carry a
    single sync-wait slot (walrus: "Too many sync wait commands")."""
    import bass_rust
    nop_id = [0]
    for fn in nc.m.functions:
        for blk in fn.blocks:
            insts = blk.instructions
            out = []
            changed = False
            for ins in insts:
                si = ins.sync_info
                is_evsem = isinstance(ins, mybir.InstEventSemaphore)
                if (si is not None and si.on_wait is not None
                        and len(si.on_wait) > 1 and not is_evsem):
                    waits = list(si.on_wait)
                    for w in waits[:-1]:
                        ev = mybir.InstEventSemaphore(
                            name=f"waitev_{nop_id[0]}", engine=ins.engine)
                        nop_id[0] += 1
                        ev.sync_info = bass_rust.SyncInfo(
                            on_wait=[w], on_update=[])
                        out.append(ev)
                    ins.sync_info = bass_rust.SyncInfo(
                        on_wait=[waits[-1]],
                        on_update=list(si.on_update or []))
                    changed = True
                out.append(ins)
            if changed:
                blk.instructions = out


def build_bass(split_waits=True):
    nc = bass.Bass()
    xT = nc.dram_tensor("xT", [C, N], BF16, kind="ExternalInput")
    wqk = nc.dram_tensor("wqk", [C, 512], BF16, kind="ExternalInput")
    bq = nc.dram_tensor("bq", [128, 4], F32, kind="ExternalInput")
    wv = nc.dram_tensor("wv", [C + 1, VWP], BF16, kind="ExternalInput")
    wo = nc.dram_tensor("wo", [2 * 128, 1024], BF16, kind="ExternalInput")
    y = nc.dram_tensor("y", [N, C], BF16, kind="ExternalOutput")
    lbounce = nc.dram_tensor("lbounce", [16, 512], F32)
    with tile.TileContext(nc) as tc:
        _emit(tc, nc, xT, wqk, bq, wv, wo, y, lbounce)
    if split_waits:
        _split_multi_waits(nc)
    return nc


def prep_core_inputs(x, w_qkv, b_qkv, w_out, core):
    """Build the per-core input arrays (bf16 for all matmul operands)."""
    import ml_dtypes
    bf = ml_dtypes.bfloat16
    b, g = divmod(core, HPC)
    heads = [HPC * g + i for i in range(HPC)]
    f = np.float32

    xTa = np.ascontiguousarray(x[b].T, f)

    def q_rows(h):
        return w_qkv[h * D:(h + 1) * D]

    def k_rows(h):
        return w_qkv[C + h * D:C + (h + 1) * D]

    def v_rows(h):
        return w_qkv[2 * C + h * D:2 * C + (h + 1) * D]

    h0, h1, h2, h3 = heads
    wqk_rows = np.concatenate([
        q_rows(h0), q_rows(h1), k_rows(h0), k_rows(h1),
        q_rows(h2), q_rows(h3), k_rows(h2), k_rows(h3)], 0)   # [512, C]
    bqk = np.concatenate([
        b_qkv[h0 * D:(h0 + 1) * D], b_qkv[h1 * D:(h1 + 1) * D],
        b_qkv[C + h0 * D:C + (h0 + 1) * D],
        b_qkv[C + h1 * D:C + (h1 + 1) * D],
        b_qkv[h2 * D:(h2 + 1) * D], b_qkv[h3 * D:(h3 + 1) * D],
        b_qkv[C + h2 * D:C + (h2 + 1) * D],
        b_qkv[C + h3 * D:C + (h3 + 1) * D]], 0)               # [512]

    wv_aug = np.zeros((C + 1, VWP), f)
    for i, h in enumerate(heads):
        wv_aug[:C, i * (D + 1):i * (D + 1) + D] = v_rows(h).T
        wv_aug[C, i * (D + 1):i * (D + 1) + D] = \
            b_qkv[2 * C + h * D:2 * C + (h + 1) * D]
        wv_aug[C, i * (D + 1) + D] = 1.0
    # pad columns so head 3's 128-col stationary slice stays in range;
    # the products land in unused psum partitions.
    wv_aug[:, VW:] = wv_aug[:, :VWP - VW]

    woa = np.concatenate([w_out[:, h * D:(h + 1) * D].T for h in heads], 0)

    return {
        "xT": np.ascontiguousarray(xTa).astype(bf),
        "wqk": np.ascontiguousarray(wqk_rows.T, f).astype(bf),
        "bq": np.ascontiguousarray(bqk.reshape(4, 128).T, f),
        "wv": np.ascontiguousarray(wv_aug).astype(bf),
        "wo": np.ascontiguousarray(woa).astype(bf),
    }


def assemble_output(partials, b_out):
    """partials: list of 8 [N, C] arrays (core order). Returns [B, N, C]."""
    y = np.empty((B, N, C), np.float32)
    for b in range(B):
        acc = partials[HPC * b].astype(np.float32)
        for g in range(1, HPC):
            acc = acc + partials[HPC * b + g].astype(np.float32)
        y[b] = acc + b_out.astype(np.float32)
    return y


_NC_CACHE = {}


def run(inputs, trace=False):
    """Returns (y_full [B,N,C] f32, exec_time_ns or None)."""
    x = np.asarray(inputs["x"], np.float32)
    w_qkv = np.asarray(inputs["w_qkv"], np.float32)
    b_qkv = np.asarray(inputs["b_qkv"], np.float32)
    w_out = np.asarray(inputs["w_out"], np.float32)
    b_out = np.asarray(inputs["b_out"], np.float32)

    if "nc" not in _NC_CACHE:
        _NC_CACHE["nc"] = build_bass()
    nc = _NC_CACHE["nc"]

    in_maps = [prep_core_inputs(x, w_qkv, b_qkv, w_out, core)
               for core in range(N_CORES)]
    res = run_bass_kernel_spmd(nc, in_maps, list(range(N_CORES)),
                               trace=trace)
    partials = [res.results[i]["y"] for i in range(N_CORES)]
    return assemble_output(partials, b_out), res.exec_time_ns


def kernel(**inputs):
    y, _ = run(inputs, trace=False)
    return y
